# revision 21
# baseline (speedup 1.0000x reference)
"""Tensor-parallel causal attention block for 8 Trainium2 NeuronCores.

Sharding: heads split across cores (2 of 16 heads per core).  Each core
computes q/k/v projections for its head slice (columns of wq/wk/wv), RoPE,
causal attention, and a row-slice of the output projection (rows of wo),
producing a partial full-shape output; the host sums the 8 partials.

All matmuls run as float32r (TF32-like, 1 cycle/row at N>=256).  Scores are
computed transposed (S^T[k, q]) so the softmax renormalization folds into
PE ones-matmuls and P needs no transposes before P@V.  V is projected
transposed (N=512, weight-stationary) and PE-transposed back to natural
layout.  All DRAM I/O uses host-blocked layouts so every DMA moves one
contiguous 256 KB tile.
"""

import math
import sys

sys.path.insert(0, "/opt/trn_rl_repo")

import numpy as np

B = 2
S = 2048
E = 2048
H = 16
D = 128
ROPE_BASE = 10000.0
NCORES = 8
HPC = H // NCORES          # heads per core = 2
DC = HPC * D               # head-dim cols per core = 256
T = B * S                  # 4096 tokens
KC = E // 128              # 16 contraction chunks
TC8 = 512                  # token chunk for projections
NTC8 = S // TC8            # 4 per batch
SB512 = 512                # query super-block
NSB = S // SB512           # 4 per batch
SCALE = 1.0 / math.sqrt(D)
LOOKAHEAD = 4              # score matmuls emitted ahead of z/sum matmuls

_COMPILED = None


def _build_program():
    import concourse.bass as bass
    import concourse.mybir as mybir
    from concourse import bacc
    from concourse.tile import TileContext

    f32 = mybir.dt.float32

    def fr(ap):
        return ap.bitcast(mybir.dt.float32r)

    nc = bacc.Bacc()
    # host-blocked layouts: every DMA tile is contiguous in DRAM
    xT_d = nc.declare_dram_parameter("xT", [KC, B * NTC8, 128, TC8], f32, isOutput=False)
    cos_d = nc.declare_dram_parameter("cosF", [128, S], f32, isOutput=False)
    sin_d = nc.declare_dram_parameter("sinF", [128, S], f32, isOutput=False)
    wq_d = nc.declare_dram_parameter("wq", [KC, 128, DC], f32, isOutput=False)
    wk_d = nc.declare_dram_parameter("wk", [KC, 128, DC], f32, isOutput=False)
    wv_d = nc.declare_dram_parameter("wv", [KC, 128, DC], f32, isOutput=False)
    wo_d = nc.declare_dram_parameter("wo", [128, HPC, E], f32, isOutput=False)
    on_d = nc.declare_dram_parameter("ones", [128, 128], f32, isOutput=False)
    id_d = nc.declare_dram_parameter("ident", [128, 128], f32, isOutput=False)
    out_d = nc.declare_dram_parameter("out", [B * (S // 128) * (E // 512), 128, 512], f32, isOutput=True)

    Exp = mybir.ActivationFunctionType.Exp
    mult = mybir.AluOpType.mult
    add = mybir.AluOpType.add

    with TileContext(nc) as tc:
        with (
            tc.tile_pool(name="wpool", bufs=1) as wp,
            tc.tile_pool(name="persist", bufs=1) as pp,
            tc.tile_pool(name="xin", bufs=6) as xp,
            tc.tile_pool(name="rope", bufs=3) as rp,
            tc.tile_pool(name="ptile", bufs=8) as ptp,
            tc.tile_pool(name="small", bufs=3) as smp,
            tc.tile_pool(name="outsb", bufs=4) as op,
        ):
            # ---- resident weights / constants ----
            wq_sb = wp.tile([128, KC, DC], f32)
            wk_sb = wp.tile([128, KC, DC], f32)
            wv_sb = wp.tile([128, KC, DC], f32)
            for kc in range(KC):
                nc.sync.dma_start(out=fr(wq_sb[:, kc, :]), in_=fr(wq_d[kc]))
                nc.sync.dma_start(out=fr(wk_sb[:, kc, :]), in_=fr(wk_d[kc]))
                nc.sync.dma_start(out=fr(wv_sb[:, kc, :]), in_=fr(wv_d[kc]))
            wo_sb = wp.tile([128, HPC, E], f32)
            nc.sync.dma_start(out=fr(wo_sb[:]), in_=fr(wo_d[:]))
            cos_sb = wp.tile([128, S], f32)
            sin_sb = wp.tile([128, S], f32)
            nc.sync.dma_start(out=cos_sb[:], in_=cos_d[:])
            nc.sync.dma_start(out=sin_sb[:], in_=sin_d[:])
            ones_sb = wp.tile([128, 128], f32)
            nc.sync.dma_start(out=fr(ones_sb[:]), in_=fr(on_d[:]))
            ident_sb = wp.tile([128, 128], f32)
            nc.sync.dma_start(out=fr(ident_sb[:]), in_=fr(id_d[:]))

            # ---- persistent per-batch arrays (slots reused across batches) ----
            qT = [pp.tile([128, S], f32, name=f"qT{h}", tag=f"qT{h}") for h in range(HPC)]
            kT = [pp.tile([128, S], f32, name=f"kT{h}", tag=f"kT{h}") for h in range(HPC)]
            v_sb = pp.tile([128, S // 128, DC], f32, name="v_sb", tag="v")
            zn = [pp.tile([128, S], f32, name=f"zn{h}", tag=f"zn{h}") for h in range(HPC)]

            for b in range(B):
                # ============ Phase A: projections + RoPE + V transpose ============
                with tc.tile_pool(name=f"psA{b}", bufs=1, space="PSUM") as pA:
                    for tc8 in range(NTC8):
                        s0 = tc8 * TC8
                        q_ps = [pA.tile([128, TC8], f32, name=f"q_ps{h}", tag=f"pq{h}") for h in range(HPC)]
                        k_ps = [pA.tile([128, TC8], f32, name=f"k_ps{h}", tag=f"pk{h}") for h in range(HPC)]
                        v_ps = [pA.tile([128, TC8], f32, name=f"v_ps{h}", tag=f"pv{h}") for h in range(HPC)]
                        for kc in range(KC):
                            xt = xp.tile([128, TC8], f32)
                            nc.sync.dma_start(out=fr(xt[:]), in_=fr(xT_d[kc, b * NTC8 + tc8]))
                            for h in range(HPC):
                                nc.tensor.matmul(q_ps[h][:], lhsT=fr(wq_sb[:, kc, h * D:(h + 1) * D]),
                                                 rhs=fr(xt[:]), start=(kc == 0), stop=(kc == KC - 1))
                                nc.tensor.matmul(k_ps[h][:], lhsT=fr(wk_sb[:, kc, h * D:(h + 1) * D]),
                                                 rhs=fr(xt[:]), start=(kc == 0), stop=(kc == KC - 1))
                                nc.tensor.matmul(v_ps[h][:], lhsT=fr(wv_sb[:, kc, h * D:(h + 1) * D]),
                                                 rhs=fr(xt[:]), start=(kc == 0), stop=(kc == KC - 1))
                        # V: copy vT out of PSUM, then PE-transpose back to natural layout
                        for h in range(HPC):
                            vt = rp.tile([128, TC8], f32, name="vt", tag="vt")
                            nc.scalar.copy(fr(vt[:]), v_ps[h][:])
                            for tb in range(TC8 // 128):
                                tp_ps = pA.tile([128, 128], f32, name="tp_ps", tag="tp", bufs=2)
                                nc.tensor.transpose(fr(tp_ps[:]), fr(vt[:, tb * 128:(tb + 1) * 128]),
                                                    fr(ident_sb[:]))
                                nc.scalar.copy(fr(v_sb[:, s0 // 128 + tb, h * D:(h + 1) * D]), tp_ps[:])
                        # RoPE drain for q/k
                        for ps_list, dst in ((q_ps, qT), (k_ps, kT)):
                            for h in range(HPC):
                                tmp = rp.tile([128, TC8], f32, name="tmp", tag="tmp")
                                nc.scalar.copy(tmp[:], ps_list[h][:])
                                rot = rp.tile([128, TC8], f32, name="rot", tag="rot")
                                nc.gpsimd.dma_start(out=rot[0:64, :], in_=tmp[64:128, :])
                                nc.gpsimd.dma_start(out=rot[64:128, :], in_=tmp[0:64, :])
                                nc.vector.tensor_tensor(tmp[:], tmp[:], cos_sb[:, s0:s0 + TC8], mult)
                                nc.vector.tensor_tensor(rot[:], rot[:], sin_sb[:, s0:s0 + TC8], mult)
                                nc.vector.tensor_tensor(fr(dst[h][:, s0:s0 + TC8]), tmp[:], rot[:], add)

                # ============ Phase B: causal attention ============
                with tc.tile_pool(name=f"psB{b}", bufs=1, space="PSUM") as pB:
                    for sb in range(NSB):
                        for h in range(HPC):
                            q_sl = qT[h][:, sb * SB512:(sb + 1) * SB512]
                            nkb = (sb + 1) * (SB512 // 128)
                            z_ps = pB.tile([128, SB512], f32, name="z_ps", tag="z", bufs=2)
                            sum_ps = pB.tile([128, SB512], f32, name="sum_ps", tag="sum", bufs=2)
                            pts = [None] * nkb

                            def emit_score(kblk):
                                st_ps = pB.tile([128, SB512], f32, name="st_ps", tag="st", bufs=4)
                                nc.tensor.matmul(st_ps[:], lhsT=fr(kT[h][:, kblk * 128:(kblk + 1) * 128]),
                                                 rhs=fr(q_sl), start=True, stop=True)
                                pt = ptp.tile([128, SB512], f32, name="pt", tag="pt")
                                nc.scalar.activation(fr(pt[:]), st_ps[:], Exp, scale=SCALE)
                                delta = kblk - sb * (SB512 // 128)
                                if delta >= 0:
                                    nc.gpsimd.affine_select(
                                        out=fr(pt[:]), in_=fr(pt[:]),
                                        pattern=[[1, SB512]], compare_op=mybir.AluOpType.is_ge,
                                        fill=0.0, base=-128 * delta, channel_multiplier=-1,
                                    )
                                pts[kblk] = pt

                            def emit_zsum(kblk):
                                pt = pts[kblk]
                                nc.tensor.matmul(z_ps[:], lhsT=fr(v_sb[:, kblk, h * D:(h + 1) * D]),
                                                 rhs=fr(pt[:]), start=(kblk == 0), stop=(kblk == nkb - 1))
                                nc.tensor.matmul(sum_ps[:], lhsT=fr(ones_sb[:]),
                                                 rhs=fr(pt[:]), start=(kblk == 0), stop=(kblk == nkb - 1))
                                pts[kblk] = None

                            for kblk in range(min(LOOKAHEAD, nkb)):
                                emit_score(kblk)
                            for kblk in range(nkb):
                                if kblk + LOOKAHEAD < nkb:
                                    emit_score(kblk + LOOKAHEAD)
                                emit_zsum(kblk)
                            rep_sb = smp.tile([128, SB512], f32, name="rep_sb", tag="repsb")
                            nc.vector.reciprocal(rep_sb[:], sum_ps[:])
                            nc.vector.tensor_tensor(fr(zn[h][:, sb * SB512:(sb + 1) * SB512]),
                                                    z_ps[:], rep_sb[:], mult)

                # ============ Phase C: output projection ============
                with tc.tile_pool(name=f"psC{b}", bufs=4, space="PSUM") as pC:
                    for tb in range(S // 128):
                        for ec in range(E // 512):
                            o_ps = pC.tile([128, 512], f32, name="o_ps", tag="o")
                            for h in range(HPC):
                                nc.tensor.matmul(o_ps[:], lhsT=fr(zn[h][:, tb * 128:(tb + 1) * 128]),
                                                 rhs=fr(wo_sb[:, h, ec * 512:(ec + 1) * 512]),
                                                 start=(h == 0), stop=(h == HPC - 1))
                            o_sb = op.tile([128, 512], f32, name="o_sb", tag="osb")
                            if ec % 2 == 0:
                                nc.vector.tensor_copy(o_sb[:], o_ps[:])
                            else:
                                nc.scalar.copy(o_sb[:], o_ps[:])
                            tile_idx = (b * (S // 128) + tb) * (E // 512) + ec
                            nc.sync.dma_start(out=out_d[tile_idx], in_=o_sb[:])



    nc.compile()
    return nc


def _get_compiled():
    global _COMPILED
    if _COMPILED is None:
        _COMPILED = _build_program()
    return _COMPILED


def _host_inputs(x, wq, wk, wv, wo):
    x = np.asarray(x, dtype=np.float32)
    # xT blocked: [KC, B*NTC8, 128, TC8]; element (kc, b*NTC8+tc8, p, c) = x[b, tc8*TC8+c, kc*128+p]
    xT = np.ascontiguousarray(
        x.transpose(2, 0, 1).reshape(KC, 128, B, NTC8, TC8).transpose(0, 2, 3, 1, 4).reshape(KC, B * NTC8, 128, TC8)
    )

    pos = np.arange(S, dtype=np.float32)
    inv_freq = (1.0 / (ROPE_BASE ** (np.arange(0, D, 2, dtype=np.float32) / np.float32(D)))).astype(np.float32)
    ang = pos[:, None] * inv_freq[None, :]          # (S, 64) fp32
    cos_h = np.cos(ang).astype(np.float32)
    sin_h = np.sin(ang).astype(np.float32)
    cosF = np.ascontiguousarray(np.concatenate([cos_h.T, cos_h.T], axis=0))   # (128, S)
    sinF = np.ascontiguousarray(np.concatenate([-sin_h.T, sin_h.T], axis=0))  # (128, S)
    ones = np.ones((128, 128), dtype=np.float32)
    ident = np.eye(128, dtype=np.float32)

    wq = np.asarray(wq, dtype=np.float32)
    wk = np.asarray(wk, dtype=np.float32)
    wv = np.asarray(wv, dtype=np.float32)
    wo = np.asarray(wo, dtype=np.float32)

    maps = []
    for c in range(NCORES):
        sl = slice(c * DC, (c + 1) * DC)
        maps.append({
            "xT": xT,
            "cosF": cosF,
            "sinF": sinF,
            "wq": np.ascontiguousarray(wq[:, sl].reshape(KC, 128, DC)),
            "wk": np.ascontiguousarray(wk[:, sl].reshape(KC, 128, DC)),
            "wv": np.ascontiguousarray(wv[:, sl].reshape(KC, 128, DC)),
            "wo": np.ascontiguousarray(wo[sl, :].reshape(HPC, 128, E).transpose(1, 0, 2)),
            "ones": ones,
            "ident": ident,
        })
    return maps


def kernel(x, wq, wk, wv, wo, _trace=False):
    from concourse.bass_utils import run_bass_kernel_spmd

    nc = _get_compiled()
    maps = _host_inputs(x, wq, wk, wv, wo)
    res = run_bass_kernel_spmd(nc, maps, list(range(NCORES)), trace=_trace)
    total = np.zeros((B * (S // 128) * (E // 512), 128, 512), dtype=np.float32)
    for c in range(NCORES):
        total += res.results[c]["out"]
    # unblock: [B, S//128, E//512, 128, 512] -> (B, S, E)
    out = np.ascontiguousarray(
        total.reshape(B, S // 128, E // 512, 128, 512).transpose(0, 1, 3, 2, 4).reshape(B, S, E)
    )
    if _trace:
        kernel.last_exec_time_ns = res.exec_time_ns
        kernel.last_trace = res.instructions_and_trace
    return out


# revision 22
# speedup vs baseline: 1.0212x; 1.0212x over previous
"""Tensor-parallel causal attention block for 8 Trainium2 NeuronCores.

Sharding: heads split across cores (2 of 16 heads per core).  Each core
computes q/k/v projections for its head slice (columns of wq/wk/wv), RoPE,
causal attention, and a row-slice of the output projection (rows of wo),
producing a partial full-shape output; the host sums the 8 partials.

All matmuls run as float32r (TF32-like, 1 cycle/row at N>=256).  Scores are
computed transposed (S^T[k, q]) so the softmax renormalization folds into
PE ones-matmuls and P needs no transposes before P@V.  V is projected
transposed (N=512, weight-stationary) and PE-transposed back to natural
layout.  All DRAM I/O uses host-blocked layouts so every DMA moves one
contiguous 256 KB tile.
"""

import math
import sys

sys.path.insert(0, "/opt/trn_rl_repo")

import numpy as np

B = 2
S = 2048
E = 2048
H = 16
D = 128
ROPE_BASE = 10000.0
NCORES = 8
HPC = H // NCORES          # heads per core = 2
DC = HPC * D               # head-dim cols per core = 256
T = B * S                  # 4096 tokens
KC = E // 128              # 16 contraction chunks
TC8 = 512                  # token chunk for projections
NTC8 = S // TC8            # 4 per batch
SB512 = 512                # query super-block
NSB = S // SB512           # 4 per batch
SCALE = 1.0 / math.sqrt(D)
LOOKAHEAD = 4              # score matmuls emitted ahead of z/sum matmuls

_COMPILED = None


def _build_program():
    import concourse.bass as bass
    import concourse.mybir as mybir
    from concourse import bacc
    from concourse.tile import TileContext

    f32 = mybir.dt.float32

    def fr(ap):
        return ap.bitcast(mybir.dt.float32r)

    nc = bacc.Bacc()
    # host-blocked layouts: every DMA tile is contiguous in DRAM
    xT_d = nc.declare_dram_parameter("xT", [KC, B * NTC8, 128, TC8], f32, isOutput=False)
    cos_d = nc.declare_dram_parameter("cosF", [128, S], f32, isOutput=False)
    sin_d = nc.declare_dram_parameter("sinF", [128, S], f32, isOutput=False)
    wq_d = nc.declare_dram_parameter("wq", [KC, 128, DC], f32, isOutput=False)
    wk_d = nc.declare_dram_parameter("wk", [KC, 128, DC], f32, isOutput=False)
    wv_d = nc.declare_dram_parameter("wv", [KC, 128, DC], f32, isOutput=False)
    wo_d = nc.declare_dram_parameter("wo", [128, HPC, E], f32, isOutput=False)
    on_d = nc.declare_dram_parameter("ones", [128, 128], f32, isOutput=False)
    id_d = nc.declare_dram_parameter("ident", [128, 128], f32, isOutput=False)
    out_d = nc.declare_dram_parameter("out", [B * (S // 128) * (E // 512), 128, 512], f32, isOutput=True)

    Exp = mybir.ActivationFunctionType.Exp
    mult = mybir.AluOpType.mult
    add = mybir.AluOpType.add

    with TileContext(nc) as tc:
        with (
            tc.tile_pool(name="wpool", bufs=1) as wp,
            tc.tile_pool(name="persist", bufs=1) as pp,
            tc.tile_pool(name="xin", bufs=6) as xp,
            tc.tile_pool(name="rope", bufs=3) as rp,
            tc.tile_pool(name="ptile", bufs=8) as ptp,
            tc.tile_pool(name="small", bufs=3) as smp,
            tc.tile_pool(name="outsb", bufs=4) as op,
        ):
            # ---- resident weights / constants ----
            wq_sb = wp.tile([128, KC, DC], f32)
            wk_sb = wp.tile([128, KC, DC], f32)
            wv_sb = wp.tile([128, KC, DC], f32)
            for kc in range(KC):
                nc.sync.dma_start(out=fr(wq_sb[:, kc, :]), in_=fr(wq_d[kc]))
                nc.sync.dma_start(out=fr(wk_sb[:, kc, :]), in_=fr(wk_d[kc]))
                nc.sync.dma_start(out=fr(wv_sb[:, kc, :]), in_=fr(wv_d[kc]))
            wo_sb = wp.tile([128, HPC, E], f32)
            nc.sync.dma_start(out=fr(wo_sb[:]), in_=fr(wo_d[:]))
            cos_sb = wp.tile([128, S], f32)
            sin_sb = wp.tile([128, S], f32)
            nc.sync.dma_start(out=cos_sb[:], in_=cos_d[:])
            nc.sync.dma_start(out=sin_sb[:], in_=sin_d[:])
            ones_sb = wp.tile([128, 128], f32)
            nc.sync.dma_start(out=fr(ones_sb[:]), in_=fr(on_d[:]))
            ident_sb = wp.tile([128, 128], f32)
            nc.sync.dma_start(out=fr(ident_sb[:]), in_=fr(id_d[:]))

            # ---- persistent per-batch arrays (slots reused across batches) ----
            qT = [pp.tile([128, S], f32, name=f"qT{h}", tag=f"qT{h}") for h in range(HPC)]
            kT = [pp.tile([128, S], f32, name=f"kT{h}", tag=f"kT{h}") for h in range(HPC)]
            v_sb = pp.tile([128, S // 128, DC], f32, name="v_sb", tag="v")
            zn = [pp.tile([128, S], f32, name=f"zn{h}", tag=f"zn{h}") for h in range(HPC)]

            for b in range(B):
                # ============ Phase A: projections + RoPE + V transpose ============
                with tc.tile_pool(name=f"psA{b}", bufs=1, space="PSUM") as pA:
                    for tc8 in range(NTC8):
                        s0 = tc8 * TC8
                        q_ps = [pA.tile([128, TC8], f32, name=f"q_ps{h}", tag=f"pq{h}") for h in range(HPC)]
                        k_ps = [pA.tile([128, TC8], f32, name=f"k_ps{h}", tag=f"pk{h}") for h in range(HPC)]
                        v_ps = [pA.tile([128, TC8], f32, name=f"v_ps{h}", tag=f"pv{h}") for h in range(HPC)]
                        for kc in range(KC):
                            xt = xp.tile([128, TC8], f32)
                            nc.sync.dma_start(out=fr(xt[:]), in_=fr(xT_d[kc, b * NTC8 + tc8]))
                            for h in range(HPC):
                                nc.tensor.matmul(q_ps[h][:], lhsT=fr(wq_sb[:, kc, h * D:(h + 1) * D]),
                                                 rhs=fr(xt[:]), start=(kc == 0), stop=(kc == KC - 1))
                                nc.tensor.matmul(k_ps[h][:], lhsT=fr(wk_sb[:, kc, h * D:(h + 1) * D]),
                                                 rhs=fr(xt[:]), start=(kc == 0), stop=(kc == KC - 1))
                                nc.tensor.matmul(v_ps[h][:], lhsT=fr(wv_sb[:, kc, h * D:(h + 1) * D]),
                                                 rhs=fr(xt[:]), start=(kc == 0), stop=(kc == KC - 1))
                        # V: copy vT out of PSUM, then PE-transpose back to natural layout
                        for h in range(HPC):
                            vt = rp.tile([128, TC8], f32, name="vt", tag="vt")
                            nc.scalar.copy(fr(vt[:]), v_ps[h][:])
                            for tb in range(TC8 // 128):
                                tp_ps = pA.tile([128, 128], f32, name="tp_ps", tag="tp", bufs=2)
                                nc.tensor.transpose(fr(tp_ps[:]), fr(vt[:, tb * 128:(tb + 1) * 128]),
                                                    fr(ident_sb[:]))
                                nc.scalar.copy(fr(v_sb[:, s0 // 128 + tb, h * D:(h + 1) * D]), tp_ps[:])
                        # RoPE drain for q/k
                        for ps_list, dst in ((q_ps, qT), (k_ps, kT)):
                            for h in range(HPC):
                                tmp = rp.tile([128, TC8], f32, name="tmp", tag="tmp")
                                nc.scalar.copy(tmp[:], ps_list[h][:])
                                rot = rp.tile([128, TC8], f32, name="rot", tag="rot")
                                nc.sync.dma_start(out=rot[0:64, :], in_=tmp[64:128, :])
                                nc.sync.dma_start(out=rot[64:128, :], in_=tmp[0:64, :])
                                nc.vector.tensor_tensor(tmp[:], tmp[:], cos_sb[:, s0:s0 + TC8], mult)
                                nc.vector.tensor_tensor(rot[:], rot[:], sin_sb[:, s0:s0 + TC8], mult)
                                nc.vector.tensor_tensor(fr(dst[h][:, s0:s0 + TC8]), tmp[:], rot[:], add)

                # ============ Phase B: causal attention ============
                with tc.tile_pool(name=f"psB{b}", bufs=1, space="PSUM") as pB:
                    for sb in range(NSB):
                        for h in range(HPC):
                            q_sl = qT[h][:, sb * SB512:(sb + 1) * SB512]
                            nkb = (sb + 1) * (SB512 // 128)
                            z_ps = pB.tile([128, SB512], f32, name="z_ps", tag="z", bufs=2)
                            sum_ps = pB.tile([128, SB512], f32, name="sum_ps", tag="sum", bufs=2)
                            pts = [None] * nkb

                            def emit_score(kblk):
                                st_ps = pB.tile([128, SB512], f32, name="st_ps", tag="st", bufs=4)
                                nc.tensor.matmul(st_ps[:], lhsT=fr(kT[h][:, kblk * 128:(kblk + 1) * 128]),
                                                 rhs=fr(q_sl), start=True, stop=True)
                                pt = ptp.tile([128, SB512], f32, name="pt", tag="pt")
                                nc.scalar.activation(fr(pt[:]), st_ps[:], Exp, scale=SCALE)
                                delta = kblk - sb * (SB512 // 128)
                                if delta >= 0:
                                    nc.gpsimd.affine_select(
                                        out=fr(pt[:]), in_=fr(pt[:]),
                                        pattern=[[1, SB512]], compare_op=mybir.AluOpType.is_ge,
                                        fill=0.0, base=-128 * delta, channel_multiplier=-1,
                                    )
                                pts[kblk] = pt

                            def emit_zsum(kblk):
                                pt = pts[kblk]
                                nc.tensor.matmul(z_ps[:], lhsT=fr(v_sb[:, kblk, h * D:(h + 1) * D]),
                                                 rhs=fr(pt[:]), start=(kblk == 0), stop=(kblk == nkb - 1))
                                nc.tensor.matmul(sum_ps[:], lhsT=fr(ones_sb[:]),
                                                 rhs=fr(pt[:]), start=(kblk == 0), stop=(kblk == nkb - 1))
                                pts[kblk] = None

                            for kblk in range(min(LOOKAHEAD, nkb)):
                                emit_score(kblk)
                            for kblk in range(nkb):
                                if kblk + LOOKAHEAD < nkb:
                                    emit_score(kblk + LOOKAHEAD)
                                emit_zsum(kblk)
                            rep_sb = smp.tile([128, SB512], f32, name="rep_sb", tag="repsb")
                            nc.vector.reciprocal(rep_sb[:], sum_ps[:])
                            nc.vector.tensor_tensor(fr(zn[h][:, sb * SB512:(sb + 1) * SB512]),
                                                    z_ps[:], rep_sb[:], mult)

                # ============ Phase C: output projection ============
                with tc.tile_pool(name=f"psC{b}", bufs=4, space="PSUM") as pC:
                    for tb in range(S // 128):
                        for ec in range(E // 512):
                            o_ps = pC.tile([128, 512], f32, name="o_ps", tag="o")
                            for h in range(HPC):
                                nc.tensor.matmul(o_ps[:], lhsT=fr(zn[h][:, tb * 128:(tb + 1) * 128]),
                                                 rhs=fr(wo_sb[:, h, ec * 512:(ec + 1) * 512]),
                                                 start=(h == 0), stop=(h == HPC - 1))
                            o_sb = op.tile([128, 512], f32, name="o_sb", tag="osb")
                            if ec % 2 == 0:
                                nc.vector.tensor_copy(o_sb[:], o_ps[:])
                            else:
                                nc.scalar.copy(o_sb[:], o_ps[:])
                            tile_idx = (b * (S // 128) + tb) * (E // 512) + ec
                            nc.sync.dma_start(out=out_d[tile_idx], in_=o_sb[:])



    nc.compile()
    return nc


def _get_compiled():
    global _COMPILED
    if _COMPILED is None:
        _COMPILED = _build_program()
    return _COMPILED


def _host_inputs(x, wq, wk, wv, wo):
    x = np.asarray(x, dtype=np.float32)
    # xT blocked: [KC, B*NTC8, 128, TC8]; element (kc, b*NTC8+tc8, p, c) = x[b, tc8*TC8+c, kc*128+p]
    xT = np.ascontiguousarray(
        x.transpose(2, 0, 1).reshape(KC, 128, B, NTC8, TC8).transpose(0, 2, 3, 1, 4).reshape(KC, B * NTC8, 128, TC8)
    )

    pos = np.arange(S, dtype=np.float32)
    inv_freq = (1.0 / (ROPE_BASE ** (np.arange(0, D, 2, dtype=np.float32) / np.float32(D)))).astype(np.float32)
    ang = pos[:, None] * inv_freq[None, :]          # (S, 64) fp32
    cos_h = np.cos(ang).astype(np.float32)
    sin_h = np.sin(ang).astype(np.float32)
    cosF = np.ascontiguousarray(np.concatenate([cos_h.T, cos_h.T], axis=0))   # (128, S)
    sinF = np.ascontiguousarray(np.concatenate([-sin_h.T, sin_h.T], axis=0))  # (128, S)
    ones = np.ones((128, 128), dtype=np.float32)
    ident = np.eye(128, dtype=np.float32)

    wq = np.asarray(wq, dtype=np.float32)
    wk = np.asarray(wk, dtype=np.float32)
    wv = np.asarray(wv, dtype=np.float32)
    wo = np.asarray(wo, dtype=np.float32)

    maps = []
    for c in range(NCORES):
        sl = slice(c * DC, (c + 1) * DC)
        maps.append({
            "xT": xT,
            "cosF": cosF,
            "sinF": sinF,
            "wq": np.ascontiguousarray(wq[:, sl].reshape(KC, 128, DC)),
            "wk": np.ascontiguousarray(wk[:, sl].reshape(KC, 128, DC)),
            "wv": np.ascontiguousarray(wv[:, sl].reshape(KC, 128, DC)),
            "wo": np.ascontiguousarray(wo[sl, :].reshape(HPC, 128, E).transpose(1, 0, 2)),
            "ones": ones,
            "ident": ident,
        })
    return maps


def kernel(x, wq, wk, wv, wo, _trace=False):
    from concourse.bass_utils import run_bass_kernel_spmd

    nc = _get_compiled()
    maps = _host_inputs(x, wq, wk, wv, wo)
    res = run_bass_kernel_spmd(nc, maps, list(range(NCORES)), trace=_trace)
    total = np.zeros((B * (S // 128) * (E // 512), 128, 512), dtype=np.float32)
    for c in range(NCORES):
        total += res.results[c]["out"]
    # unblock: [B, S//128, E//512, 128, 512] -> (B, S, E)
    out = np.ascontiguousarray(
        total.reshape(B, S // 128, E // 512, 128, 512).transpose(0, 1, 3, 2, 4).reshape(B, S, E)
    )
    if _trace:
        kernel.last_exec_time_ns = res.exec_time_ns
        kernel.last_trace = res.instructions_and_trace
    return out
